# revision 59
# baseline (speedup 1.0000x reference)
"""Trainium2 Bass kernel for a 2-layer GCN encoder (PyG GCNConv semantics).

  out = A_hat @ (relu(A_hat @ (x @ W1) + b1) @ W2) + b2
  A_hat = D^-1/2 (A + I) D^-1/2,  deg computed on dst (col) with self loops.

Strategy (8 NeuronCores, SPMD, node/dst sharding), v2:
  - Layer 2 is restructured as out = (A_hat @ h) @ W2 + b2: the layer-2
    gather table is h (dis-prescaled relu output, 128 cols bf16 = 256B rows)
    produced directly by the L1 evacuation chain (PE transpose + ACT scale),
    so the a-half AllGather of the L2 table fires mid-way through L1's
    propagate and the inter-layer bubble mostly disappears.  The @W2 and +b2
    run per dst tile fused into the L2 evacuation.
  - SWDGE descriptor generation (~8ns/idx per call on a Q7 cpu pair) is the
    critical resource; calls on different SWDGE queues overlap, so each
    chunk-half gather is split into two subcalls round-robined over all 4
    queues to force ~4-way generation concurrency.
  - Scatter one-hots S[e,d] are host-built, stored fp8e4 (exact 0/1) and
    streamed on the HWDGE path; PE matmul takes fp8 rhs against bf16 lhsT.
  - Separate PSUM pools for chunk accumulation vs evac transpose/final
    matmuls (a shared pool serialized the v1 pipeline).
  - Host does index prep + layout/dtype conversion only.
"""

import math
import os
import sys
import types

import numpy as np
import ml_dtypes

import concourse.bacc as bacc
import concourse.bass as bass
import concourse.mybir as mybir
import concourse.tile as tile
from concourse import bass_utils


def _install_ntff_hook():
    """Bridge the missing antenv.axon_hooks so trace=True works under axon."""
    try:
        if "antenv.axon_hooks" in sys.modules:
            return
        import trn_agent_boot.trn_boot as tb

        hook = tb._ntff_profile_via_ctypes("/opt/axon/libaxon_pjrt.so")
        mod = types.ModuleType("antenv.axon_hooks")
        mod.get_axon_ntff_profile_hook = lambda: hook
        mod.set_axon_ntff_profile_hook = lambda h: None
        sys.modules["antenv.axon_hooks"] = mod
        import antenv

        antenv.axon_hooks = mod
        bass_utils.upload_artifacts = lambda tmpdir: tmpdir
    except Exception:
        pass

P = 128
BF16 = ml_dtypes.bfloat16
FP8 = ml_dtypes.float8_e4m3fn

FULL_CFG = dict(N=50000, E=800000, IN=500, H=128, OUT=64, NC=8, CHUNK=4)

LAST_RESULTS = None  # test harness reads exec_time_ns from here


# --------------------------------------------------------------------------
# Host-side preprocessing (index manipulation + input layout only)
# --------------------------------------------------------------------------


def _ceil_to(a, m):
    return (a + m - 1) // m * m


def _wrap16(idx):
    """[G] int16 -> [128, G//16]: edge j at partition j%16 slot j//16, x8 replicated."""
    g = idx.shape[0]
    w = idx.reshape(g // 16, 16).T
    return np.ascontiguousarray(np.tile(w, (8, 1)))


def _wrap128(v):
    """[G] -> [128, G//128]: edge j at partition j%128 slot j//128."""
    g = v.shape[0]
    return np.ascontiguousarray(v.reshape(g // P, P).T)


def _plan_and_prep(x, edge_index, W1, b1, W2, b2, cfg):
    N, E, IN, H, OUT, NC = (
        cfg["N"], cfg["E"], cfg["IN"], cfg["H"], cfg["OUT"], cfg["NC"],
    )
    CHUNK = cfg["CHUNK"]
    NL = N // NC                      # real nodes per core
    NLP = _ceil_to(NL, P)             # padded nodes per core
    TPC = NLP // P                    # dst tiles per core
    NP = NLP * NC                     # padded global nodes
    INP = _ceil_to(IN, P)             # padded input feature dim
    KC = INP // P                     # k chunks for x @ W1
    # split each shard into tile-aligned halves a/b so both gather windows
    # fit int16 and the two AllGathers pipeline
    TA = (TPC + 1) // 2               # a-half tiles per core
    TB = TPC - TA
    SA, SB = TA * P, TB * P           # a/b rows per core
    NPA, NPB = SA * NC, SB * NC       # table rows
    assert NPA < 32768 and NPB < 32768

    # ---- graph WITHOUT self loops (diag handled on-device) ----
    src = edge_index[0].astype(np.int64)
    dst = edge_index[1].astype(np.int64)
    in_deg = np.bincount(dst, minlength=N)
    deg = in_deg.astype(np.float32) + 1.0  # + self loop
    dis = (1.0 / np.sqrt(deg)).astype(np.float32)

    # Degree-balanced relabeling: deal nodes round-robin (by in-degree rank)
    # across all NP//P global tiles so per-(core,tile) edge counts equalize
    # and the max-over-cores tile padding shrinks.
    r = np.arange(N, dtype=np.int64)
    slot_of = (r // NL) * NLP + r % NL

    sp = slot_of[src]                 # padded global src slot
    dp = slot_of[dst]                 # padded global dst slot
    core = dp // NLP                  # owning core of dst
    dloc = dp % NLP                   # local dst id
    t_of = dloc // P                  # dst tile within core
    dint = (dloc % P).astype(np.float32)  # dst id within tile
    s_core = sp // NLP                # owning core of src
    s_loc = sp % NLP                  # local src slot
    half = (s_loc >= SA).astype(np.int64)  # 0 = a table, 1 = b table
    win_idx = np.where(half == 0, s_core * SA + s_loc, s_core * SB + (s_loc - SA))
    assert win_idx.max() < 32768

    # ---- packed chunk layout: groups butt-joined, pad per (chunk, half) ----
    # Group (t, h) occupies an UNALIGNED slot run of length L = max-over-core
    # count; only each (chunk, half) gather call is padded to 128.  An edge-
    # tile window straddling two dst groups gets one masked S-tile (and one
    # matmul) per group, so ~10% of gather rows and S bytes disappear at the
    # cost of ~+10 matmuls per chunk.
    gid = (core * TPC + t_of) * 2 + half
    cnt = np.bincount(gid, minlength=NC * TPC * 2).reshape(NC, TPC, 2)
    L = cnt.max(axis=0).astype(np.int64)          # [TPC, 2] packed group len

    nchunks = math.ceil(TPC / CHUNK)
    chunk_tiles = [list(range(c * CHUNK, min((c + 1) * CHUNK, TPC))) for c in range(nchunks)]
    off = np.zeros((TPC, 2), np.int64)            # slot offset of group
    calls = []       # [ci][h] -> (slot_base, n_slots)  (both 128-mult)
    winfo = []       # [ci] -> (window_base, n_windows)
    chunk_scols = [] # [ci] -> (s_col_base, n_s_tiles)
    t_wins = [[] for _ in range(TPC)]  # per t: [(window, s_col), ...] a then b
    sidx_pairs = []
    pos = 0
    s_cols = 0
    for ci, tlist in enumerate(chunk_tiles):
        wbase = pos // P
        sbase = s_cols
        percall = []
        for h in (0, 1):
            cb = pos
            for t in tlist:
                off[t, h] = pos
                pos += L[t, h]
            pos = _ceil_to(pos, P)
            percall.append((cb, pos - cb))
            for t in tlist:
                if L[t, h] == 0:
                    continue
                w0 = off[t, h] // P
                w1 = (off[t, h] + L[t, h] - 1) // P
                for w in range(w0, w1 + 1):
                    sidx_pairs.append((t, w, s_cols))
                    t_wins[t].append((w, s_cols))
                    s_cols += 1
        calls.append(percall)
        winfo.append((wbase, pos // P - wbase))
        chunk_scols.append((sbase, s_cols - sbase))
    GP = pos
    total_windows = GP // P

    # ---- per-core packed edge arrays (sorted by src within groups) ----
    order = np.lexsort((win_idx, half, t_of, core))
    gid_sorted = gid[order]
    first = np.ones(len(order), bool)
    first[1:] = gid_sorted[1:] != gid_sorted[:-1]
    group_start = np.where(first)[0]
    start_of = np.zeros(NC * TPC * 2, np.int64)
    start_of[gid_sorted[group_start]] = group_start
    rank = np.arange(len(order)) - start_of[gid_sorted]
    slot = off[t_of[order], half[order]] + rank

    idx16 = np.zeros((NC, GP), np.int16)
    c_sorted = core[order]
    idx16[c_sorted, slot] = win_idx[order].astype(np.int16)

    # subcall split at window granularity within each (chunk, half) call:
    # 2 pieces per half, round-robined over the 4 SWDGE queues
    subcalls = []  # [ci][h] -> list of (window_off_within_half, n_windows)
    for ci in range(nchunks):
        percall = []
        for h in (0, 1):
            nw = calls[ci][h][1] // P
            n1 = nw // 2
            percall.append([(0, n1), (n1, nw - n1)] if n1 else ([(0, nw)] if nw else []))
        subcalls.append(percall)

    # host-built one-hot scatter matrices, one masked S-tile per (t, window):
    # S_dram[c][e, s_col*128 + d] = (slot e of window belongs to t with lane d)
    lut = np.full((TPC, total_windows), -1, np.int64)
    for t, w, si in sidx_pairs:
        lut[t, w] = si
    e_w = slot // P
    e_l = slot % P
    sidx_of = lut[t_of[order], e_w]
    assert (sidx_of >= 0).all()
    s_dram = np.zeros((NC, P, s_cols * P), FP8)
    s_dram[c_sorted, e_l, sidx_of * P + dint[order].astype(np.int64)] = 1.0

    # ---- per-core dense inputs ----
    x = np.asarray(x, np.float32)
    W1p = np.zeros((INP, H), np.float32)
    W1p[:IN] = np.asarray(W1, np.float32)
    ident = np.eye(P, dtype=np.float32)
    b2rep = np.ascontiguousarray(
        np.tile(np.asarray(b2, np.float32).reshape(1, OUT), (P, 1))
    )
    b1rep = np.ascontiguousarray(
        np.tile(np.asarray(b1, np.float32).reshape(1, H), (P, 1))
    )

    xT_all = np.zeros((INP, NP), np.float32)
    xT_all[:IN, slot_of] = x.T
    disl_all = np.zeros(NP, np.float32)
    disl_all[slot_of] = dis

    in_maps = []
    for c in range(NC):
        xT = xT_all[:, c * NLP : (c + 1) * NLP]
        disl = disl_all[c * NLP : (c + 1) * NLP]
        in_maps.append(
            {
                "xT": xT.astype(BF16),
                "w1": W1p.astype(BF16),
                "w2": np.asarray(W2, np.float32).astype(BF16),
                "b1rep": b1rep,
                "b2rep": b2rep,
                "ident": ident.astype(BF16),
                "dis_local": _wrap128(disl),
                "dis_rep": np.ascontiguousarray(np.tile(disl.reshape(1, NLP), (P, 1))),
                "idx": _wrap16(idx16[c]),
                "s_mat": np.ascontiguousarray(s_dram[c]),
            }
        )

    plan = dict(
        cfg=cfg, NL=NL, NLP=NLP, TPC=TPC, NP=NP, INP=INP, KC=KC, slot_of=slot_of,
        TA=TA, TB=TB, SA=SA, SB=SB, NPA=NPA, NPB=NPB,
        GP=GP, s_cols=s_cols, calls=calls, winfo=winfo,
        chunk_scols=chunk_scols, t_wins=t_wins,
        chunk_tiles=chunk_tiles, subcalls=subcalls,
    )
    return plan, in_maps


# --------------------------------------------------------------------------
# Device program
# --------------------------------------------------------------------------


def _build_program(plan):
    cfg = plan["cfg"]
    N, IN, H, OUT, NC = cfg["N"], cfg["IN"], cfg["H"], cfg["OUT"], cfg["NC"]
    CHUNK = cfg["CHUNK"]
    NLP, TPC, NP, INP, KC, GP = (
        plan["NLP"], plan["TPC"], plan["NP"], plan["INP"], plan["KC"], plan["GP"],
    )
    TA, TB, SA, SB, NPA, NPB = (
        plan["TA"], plan["TB"], plan["SA"], plan["SB"], plan["NPA"], plan["NPB"],
    )
    calls, winfo = plan["calls"], plan["winfo"]
    chunk_scols, t_wins = plan["chunk_scols"], plan["t_wins"]
    f32 = mybir.dt.float32
    bf16 = mybir.dt.bfloat16
    fp8 = mybir.dt.float8e4

    nc = bacc.Bacc("TRN2", target_bir_lowering=False, debug=False, num_swdge_queues=4)

    xT_d = nc.dram_tensor("xT", [INP, NLP], bf16, kind="ExternalInput")
    w1_d = nc.dram_tensor("w1", [INP, H], bf16, kind="ExternalInput")
    w2_d = nc.dram_tensor("w2", [H, OUT], bf16, kind="ExternalInput")
    b1rep_d = nc.dram_tensor("b1rep", [P, H], f32, kind="ExternalInput")
    b2rep_d = nc.dram_tensor("b2rep", [P, OUT], f32, kind="ExternalInput")
    ident_d = nc.dram_tensor("ident", [P, P], bf16, kind="ExternalInput")
    disl_d = nc.dram_tensor("dis_local", [P, TPC], f32, kind="ExternalInput")
    disrep_d = nc.dram_tensor("dis_rep", [P, NLP], f32, kind="ExternalInput")
    idx_d = nc.dram_tensor("idx", [P, GP // 16], mybir.dt.int16, kind="ExternalInput")
    smat_d = nc.dram_tensor(
        "s_mat", [P, plan["s_cols"] * P], fp8, kind="ExternalInput"
    )

    t1_loc = [
        nc.dram_tensor("t1_local_a", [SA, H], bf16),
        nc.dram_tensor("t1_local_b", [SB, H], bf16),
    ]
    t1_tab = [
        nc.dram_tensor("t1_tab_a", [NPA, H], bf16, addr_space="Shared"),
        nc.dram_tensor("t1_tab_b", [NPB, H], bf16, addr_space="Shared"),
    ]
    t2_loc = [
        nc.dram_tensor("t2_local_a", [SA, H], bf16),
        nc.dram_tensor("t2_local_b", [SB, H], bf16),
    ]
    t2_tab = [
        nc.dram_tensor("t2_tab_a", [NPA, H], bf16, addr_space="Shared"),
        nc.dram_tensor("t2_tab_b", [NPB, H], bf16, addr_space="Shared"),
    ]
    out_d = nc.dram_tensor("out_local", [NLP, OUT], f32, kind="ExternalOutput")

    nchunks = len(plan["chunk_tiles"])
    ca_last = (TA - 1) // CHUNK       # last chunk containing a-half dst tiles
    assert TA % CHUNK == 0 or ca_last == nchunks - 1 or True

    with tile.TileContext(nc) as tc:
        with (
            tc.tile_pool(name="const", bufs=1) as const_pool,
            tc.tile_pool(name="stage", bufs=1) as stage_pool,
        ):
            # ---- persistent SBUF tiles --------------------------------
            ident_sb = const_pool.tile([P, P], bf16)
            nc.sync.dma_start(ident_sb[:], ident_d[:])
            b1rep_sb = const_pool.tile([P, H], f32)
            nc.sync.dma_start(b1rep_sb[:], b1rep_d[:])
            b2rep_sb = const_pool.tile([P, OUT], f32)
            nc.sync.dma_start(b2rep_sb[:], b2rep_d[:])
            disl_sb = const_pool.tile([P, TPC], f32)
            nc.sync.dma_start(disl_sb[:], disl_d[:])
            w2_sb = const_pool.tile([H, OUT], bf16)
            nc.sync.dma_start(w2_sb[:], w2_d[:])
            idx_sb = const_pool.tile([P, GP // 16], mybir.dt.int16)
            nc.sync.dma_start(idx_sb[:], idx_d[:])
            disrep_sb = const_pool.tile([P, NLP], f32)
            nc.sync.dma_start(disrep_sb[:], disrep_d[:])

            t1_stage = stage_pool.tile([P, TPC, H], bf16)
            t2_stage = stage_pool.tile([P, TPC, H], bf16)

            def allgather(loc, tab):
                nc.gpsimd.collective_compute(
                    "AllGather",
                    mybir.AluOpType.bypass,
                    replica_groups=[list(range(NC))],
                    ins=[loc[:]],
                    outs=[tab[:]],
                )

            # ============ Phase A: t1 = dis * (x @ W1) ==================
            with (
                tc.tile_pool(name="xa", bufs=1) as xa_pool,
                tc.tile_pool(name="pa", bufs=4, space="PSUM") as pa_psum,
            ):
                w1_sb = xa_pool.tile([P, KC, H], bf16)
                nc.sync.dma_start(w1_sb[:], w1_d.rearrange("(k p) h -> p k h", p=P))
                # split the x load per half so a-tile matmuls (and thus the
                # first AllGather) start before the whole 6.25MB lands
                xk_a = xa_pool.tile([P, KC, SA], bf16)
                nc.sync.dma_start(
                    xk_a[:], xT_d.rearrange("(k p) n -> p k n", p=P)[:, :, 0:SA]
                )
                xk_b = xa_pool.tile([P, KC, SB], bf16)
                nc.sync.dma_start(
                    xk_b[:], xT_d.rearrange("(k p) n -> p k n", p=P)[:, :, SA:NLP]
                )

                for h, t0, t1_, loc in ((0, 0, TA, t1_loc[0]), (1, TA, TPC, t1_loc[1])):
                    xk = xk_a if h == 0 else xk_b
                    toff = t0 * P
                    for t in range(t0, t1_):
                        ps = pa_psum.tile([P, H], f32, space="PSUM")
                        for k in range(KC):
                            nc.tensor.matmul(
                                out=ps[:],
                                lhsT=xk[:, k, t * P - toff : (t + 1) * P - toff],
                                rhs=w1_sb[:, k, :],
                                start=(k == 0),
                                stop=(k == KC - 1),
                            )
                        nc.scalar.activation(
                            out=t1_stage[:, t, :],
                            in_=ps[:],
                            func=mybir.ActivationFunctionType.Copy,
                            bias=0.0,
                            scale=disl_sb[:, t : t + 1],
                        )

            # staging + AllGathers OUTSIDE the xa/pa pool scope: the pool
            # teardown barrier must not make downstream pools (whose SBUF
            # ranges reuse xa's) wait for collective completion.
            for h, t0, t1_, loc in ((0, 0, TA, t1_loc[0]), (1, TA, TPC, t1_loc[1])):
                nc.scalar.dma_start(
                    loc.rearrange("(t p) h -> p t h", p=P),
                    t1_stage[:, t0:t1_, :],
                )
                allgather(loc, t1_tab[h])

            # ============ Propagate (both layers) =======================
            max_chunk_tiles = max(nw for _, nw in winfo)
            max_chunk_scols = max(ns for _, ns in chunk_scols)
            with (
                tc.tile_pool(name="msg", bufs=5) as msg_pool,
                tc.tile_pool(name="s", bufs=5) as s_pool,
                tc.tile_pool(name="ev", bufs=4) as ev_pool,
                tc.tile_pool(name="hh", bufs=7) as hh_pool,
                tc.tile_pool(name="ps", bufs=6, space="PSUM") as ps_psum,
                tc.tile_pool(name="tr", bufs=2, space="PSUM") as tr_psum,
            ):
                qctr = [0]
                # one persistent GPR per distinct num_idxs value: per-call
                # to_reg reuses a freed physical reg, creating a WAR hazard
                # that serializes consecutive dma_gather calls.
                sizereg = {}

                def reg_of(n):
                    if n not in sizereg:
                        sizereg[n] = nc.gpsimd.to_reg(n)
                    return sizereg[n]

                def gather_calls(tabs, msg, ci, wbase):
                    """Per chunk: ~2 subcalls per half, round-robin all queues."""
                    for h in (0, 1):
                        cb = calls[ci][h][0]
                        for st, sn in plan["subcalls"][ci][h]:
                            if not sn:
                                continue
                            nidx = sn * P
                            base = cb // P + st
                            lo = base - wbase
                            nc.gpsimd.dma_gather(
                                msg[:, lo : lo + sn, :],
                                tabs[h][:],
                                idx_sb[:, base * 8 : (base + sn) * 8],
                                nidx,
                                reg_of(nidx),
                                H,
                                single_packet=False,
                                queue_num=qctr[0] % 4,
                            )
                            qctr[0] += 1

                def propagate(tabs, stage_sb, evac1_cb, evac2_cb, flip,
                              chunk_done_cb=None):
                    for ci, tlist in enumerate(plan["chunk_tiles"]):
                        wbase, nw = winfo[ci]
                        sbase, nsc = chunk_scols[ci]
                        if nw == 0:
                            continue
                        msg = msg_pool.tile([P, max_chunk_tiles, H], bf16, tag="msg")
                        # one-hot block stream: sync queue carries ONLY these,
                        # so its pool-slot waits never block compute queues
                        s_chunk = s_pool.tile([P, max_chunk_scols * P], fp8, tag="s")
                        nc.sync.dma_start(
                            s_chunk[:, : nsc * P],
                            smat_d[:, sbase * P : (sbase + nsc) * P],
                        )
                        gather_calls(tabs, msg, ci, wbase)

                        # pass 1: all accumulation matmuls + off-PE evac
                        evd = {}
                        for t in tlist:
                            wins = t_wins[t]
                            ps = ps_psum.tile([P, P], f32, space="PSUM", tag="ps")
                            for j, (w, sc) in enumerate(wins):
                                k = w - wbase
                                sk = sc - sbase
                                if flip:
                                    # psum[d, f] += S[e, d]^T @ msg[e, f]
                                    nc.tensor.matmul(
                                        out=ps[:],
                                        lhsT=s_chunk[:, sk * P : (sk + 1) * P],
                                        rhs=msg[:, k, :],
                                        start=(j == 0),
                                        stop=False,
                                    )
                                else:
                                    # psum[f, d] += msg[e, f]^T @ S[e, d]
                                    nc.tensor.matmul(
                                        out=ps[:],
                                        lhsT=msg[:, k, :],
                                        rhs=s_chunk[:, sk * P : (sk + 1) * P],
                                        start=(j == 0),
                                        stop=False,
                                    )
                            # diagonal (self-loop) term: + dis_d * table_row[d]
                            if flip:
                                nc.tensor.matmul(
                                    out=ps[:],
                                    lhsT=ident_sb[:],
                                    rhs=stage_sb[:, t, :],
                                    start=(not wins),
                                    stop=True,
                                )
                            else:
                                nc.tensor.matmul(
                                    out=ps[:],
                                    lhsT=stage_sb[:, t, :],
                                    rhs=ident_sb[:],
                                    start=(not wins),
                                    stop=True,
                                )
                            evd[t] = evac1_cb(t, ps)
                        # pass 2: PE-dependent evac, batched so the PE queue
                        # never waits on a fresh DVE/ACT chain
                        for t in tlist:
                            evac2_cb(t, evd[t])
                        if chunk_done_cb is not None:
                            chunk_done_cb(ci)

                # ---- L1 evac (flipped psum [d, f], node-major) ----
                # t2[d, f] = dis_d * relu(dis_d * psum + b1)
                #          = relu(dis_d * (dis_d * psum + b1))   (dis_d > 0)
                def l1_evac1(t, ps):
                    tmp = ev_pool.tile([P, P], f32, tag="ev1")
                    nc.vector.tensor_scalar(
                        out=tmp[:],
                        in0=ps[:],
                        scalar1=disl_sb[:, t : t + 1],
                        scalar2=None,
                        op0=mybir.AluOpType.mult,
                    )
                    tmp2 = ev_pool.tile([P, P], f32, tag="ev1b")
                    nc.vector.tensor_tensor(
                        out=tmp2[:],
                        in0=tmp[:],
                        in1=b1rep_sb[:],
                        op=mybir.AluOpType.add,
                    )
                    nc.scalar.activation(
                        out=t2_stage[:, t, :],
                        in_=tmp2[:],
                        func=mybir.ActivationFunctionType.Relu,
                        bias=0.0,
                        scale=disl_sb[:, t : t + 1],
                    )
                    return None

                def l1_evac2(t, _):
                    pass

                def l1_chunk_done(ci):
                    # a-half of the h table complete -> stage + AllGather now
                    if ci == ca_last:
                        nc.scalar.dma_start(
                            t2_loc[0].rearrange("(t p) h -> p t h", p=P),
                            t2_stage[:, 0:TA, :],
                        )
                        allgather(t2_loc[0], t2_tab[0])

                propagate(t1_tab, t1_stage, l1_evac1, l1_evac2, True, l1_chunk_done)

                # b-half of h table
                nc.scalar.dma_start(
                    t2_loc[1].rearrange("(t p) h -> p t h", p=P),
                    t2_stage[:, TA:TPC, :],
                )
                allgather(t2_loc[1], t2_tab[1])

                # ---- L2 evac: out = (dis*psum) @ W2 + b2 ----
                outT = out_d.rearrange("(t p) h -> p t h", p=P)

                def l2_evac1(t, ps):
                    tmp = ev_pool.tile([P, P], f32, tag="ev2")
                    nc.vector.tensor_tensor(
                        out=tmp[:],
                        in0=ps[:],
                        in1=disrep_sb[:, t * P : (t + 1) * P],
                        op=mybir.AluOpType.mult,
                    )
                    gs = hh_pool.tile([P, P], bf16, tag="gs")
                    nc.scalar.activation(
                        out=gs[:],
                        in_=tmp[:],
                        func=mybir.ActivationFunctionType.Copy,
                        bias=0.0,
                        scale=1.0,
                    )
                    return gs

                def l2_evac2(t, gs):
                    ps2 = tr_psum.tile([P, OUT], f32, space="PSUM", tag="ps2")
                    nc.tensor.matmul(
                        out=ps2[:], lhsT=gs[:], rhs=w2_sb[:], start=True, stop=True
                    )
                    out_t = ev_pool.tile([P, OUT], f32, tag="outt")
                    nc.vector.tensor_tensor(
                        out=out_t[:],
                        in0=ps2[:],
                        in1=b2rep_sb[:],
                        op=mybir.AluOpType.add,
                    )
                    nc.scalar.dma_start(outT[:, t, :], out_t[:])

                propagate(t2_tab, t2_stage, l2_evac1, l2_evac2, False)

    nc.compile()
    return nc


# --------------------------------------------------------------------------
# Entry point
# --------------------------------------------------------------------------


def _run(inputs, cfg=None, trace=False):
    global LAST_RESULTS
    cfg = dict(FULL_CFG if cfg is None else cfg)
    plan, in_maps = _plan_and_prep(
        inputs["x"], inputs["edge_index"], inputs["W1"], inputs["b1"],
        inputs["W2"], inputs["b2"], cfg,
    )
    nc = _build_program(plan)
    if trace:
        _install_ntff_hook()
    res = bass_utils.run_bass_kernel_spmd(
        nc, in_maps, core_ids=list(range(cfg["NC"])), trace=trace
    )
    LAST_RESULTS = res
    out_padded = np.concatenate(
        [res.results[c]["out_local"] for c in range(cfg["NC"])], axis=0
    )
    return out_padded[plan["slot_of"]].astype(np.float32)


def kernel(**inputs):
    return _run(inputs, trace=bool(os.environ.get("GCN_TRACE")))
